# revision 1
# baseline (speedup 1.0000x reference)
"""BestRQ loss kernel for 8 Trainium2 NeuronCores.

Math notes (all exact reformulations of the reference):
  - loss = sum_t m_t * ce_t / (sum(m)*C) with m = pad & masked, C = 1.
  - At every token with m_t = 1, masked_xs_t == mask_emb exactly, so
    logits_t == L0 := mask_emb @ W (one shared [N] row) and
    logsumexp(logits_t) == S0 := logsumexp(L0) (one shared scalar).
    => loss = S0 - (sum_t m_t * L0[target_t]) / sum(m).
    The whole [B,T,N] logits tensor / softmax is unnecessary.
  - target_t = argmin_n dist = argmax_n score_tn,
    score_tn = proj_t . emb_n - 0.5*|emb_n|^2.
  - L0[target_t] is extracted without computing any argmax index:
        maxs_t = max_n score_tn            (K=32 matmul)
        maxv_t = max_n (score_tn + delta*L0_n)   (K=33 matmul, same prefix)
        L0[target_t] ~= (maxv_t - maxs_t) / delta
    Both matmuls share the same fp32 accumulation prefix over rows 0..31,
    so the subtraction is Sterbenz-exact up to one ulp of score.
  - Score matmul inputs are bf16 (PE fp32 runs LOW_HIGH at half rate and
    was HAM-throttled); the resulting ~0.03 absolute score noise flips
    near-tied argmaxes for a few % of tokens, but each flip substitutes a
    near-equivalent codeword whose L0 differs by ~0.05 with random sign,
    and the loss averages ~4096 tokens -> ~2e-5 relative error.
  - Only masked tokens matter, so the host gathers the ~4096 masked token
    positions, splits them across the 8 cores and pads to a static shape.
    Padded slots carry m=0 and contribute exactly zero.
"""

import numpy as np

try:
    import concourse.bass as bass  # noqa: F401
except ImportError:  # pragma: no cover
    import sys

    sys.path.insert(0, "/opt/trn_rl_repo")
    import concourse.bass as bass  # noqa: F401

import concourse.mybir as mybir
from concourse import bacc, bass_utils, masks
from concourse.tile import TileContext

F32 = mybir.dt.float32
BF16 = mybir.dt.bfloat16
U8 = mybir.dt.uint8

B, T, D, E, N = 16, 512, 256, 16, 8192
NCORES = 8
EPS = 1e-5
DELTA = 1e-2

NT = 5          # token tiles per core (5*128 = 640 slots >= worst-case masked count)
TOK = NT * 128
BLK = 1024      # psum score-block width (2 banks, one bf16 matmul)
NBLK = N // BLK
BETA = 2000.0   # sharpness of the exp-moment argmax extraction

_CACHE = {}


def _build_bass():
    nc = bacc.Bacc(
        "TRN2", target_bir_lowering=False, debug=False, num_devices=NCORES
    )
    xs = nc.dram_tensor("xs", [TOK, D], F32, kind="ExternalInput")
    pm = nc.dram_tensor("pm", [TOK], U8, kind="ExternalInput")
    mm = nc.dram_tensor("mm", [TOK], U8, kind="ExternalInput")
    gamma = nc.dram_tensor("gamma", [D], F32, kind="ExternalInput")
    beta = nc.dram_tensor("beta", [D], F32, kind="ExternalInput")
    projw = nc.dram_tensor("projw", [D, E], F32, kind="ExternalInput")
    emb = nc.dram_tensor("emb", [E, N], F32, kind="ExternalInput")
    wmat = nc.dram_tensor("wmat", [D, N], F32, kind="ExternalInput")
    maske = nc.dram_tensor("maske", [D], F32, kind="ExternalInput")
    out = nc.dram_tensor("out", [3, 1], F32, kind="ExternalOutput")

    AX = mybir.AxisListType.X
    OP = mybir.AluOpType
    AF = mybir.ActivationFunctionType

    with TileContext(nc) as tc:
        with (
            tc.tile_pool(name="const", bufs=1) as cst,
            tc.tile_pool(name="embp", bufs=1) as embp,
            tc.tile_pool(name="wp", bufs=4) as wp,
            tc.tile_pool(name="xsp", bufs=2) as xsp,
            tc.tile_pool(name="work", bufs=2) as wk,
            tc.tile_pool(name="small", bufs=4) as sm,
            tc.tile_pool(name="vsb", bufs=3) as vsbp,
            tc.tile_pool(name="psc", bufs=3, space="PSUM") as psc,
            tc.tile_pool(name="psm", bufs=2, space="PSUM") as psm,
        ):
            # ---------------- constants / setup ----------------
            ident = cst.tile([128, 128], F32)
            masks.make_identity(nc, ident[:])

            ones128 = cst.tile([128, 1], F32)
            nc.vector.memset(ones128[:], 1.0)

            scale2 = cst.tile([2, 1], F32)
            nc.vector.memset(scale2[:], 1.0)
            nc.vector.tensor_scalar(
                scale2[0:1, :], scale2[0:1, :], 1.0 / (DELTA * BETA), None,
                op0=OP.mult,
            )

            epsb = cst.tile([128, 1], F32)
            nc.vector.memset(epsb[:], EPS)

            # gamma/beta/mask_emb as two 128-row chunks
            gam = cst.tile([128, 2], F32)
            bet = cst.tile([128, 2], F32)
            mke = cst.tile([128, 2], F32)
            nc.sync.dma_start(gam[:], gamma.rearrange("(a b) -> b a", b=128))
            nc.sync.dma_start(bet[:], beta.rearrange("(a b) -> b a", b=128))
            nc.sync.dma_start(mke[:], maske.rearrange("(a b) -> b a", b=128))

            # projection, gamma-folded: Pp[:, kc, :] = gamma_chunk * P_chunk
            praw = cst.tile([128, 2, E], F32)
            nc.sync.dma_start(praw[:], projw.rearrange("(a b) e -> b a e", b=128))
            pp = cst.tile([128, 2, E], F32)
            for kc in range(2):
                nc.vector.tensor_scalar(
                    pp[:, kc, :], praw[:, kc, :], gam[:, kc : kc + 1], None,
                    op0=OP.mult,
                )

            # b0T = Pp^T beta  [E,1]
            b0ps = psm.tile([E, 1], F32, tag="misc")
            for kc in range(2):
                nc.tensor.matmul(
                    b0ps[:], pp[:, kc, :], bet[:, kc : kc + 1],
                    start=(kc == 0), stop=(kc == 1),
                )
            b0t = cst.tile([E, 1], F32)
            nc.vector.tensor_copy(b0t[:], b0ps[:])

            # em3b (bf16): rows 0:16 emb, 16:32 emb^2, 32 delta*L0
            em3f = embp.tile([16, N], F32)
            nc.sync.dma_start(em3f[:], emb[:, :])
            em3b = embp.tile([33, N], BF16)
            nc.vector.tensor_copy(em3b[0:16, :], em3f[:])
            sq16 = embp.tile([16, N], BF16)
            nc.scalar.activation(sq16[:], em3f[:], AF.Square)
            nc.sync.dma_start(em3b[16:32, :], sq16[:])

            # delta*L0 row via W stream (fp32); matmul lands on partition 32
            for ncx in range(16):
                sl = slice(ncx * 512, (ncx + 1) * 512)
                l0ps = psm.tile([33, 512], F32, tag="misc")
                for kc in range(2):
                    wt = wp.tile([128, 512], F32)
                    nc.sync.dma_start(
                        wt[:], wmat[kc * 128 : (kc + 1) * 128, sl]
                    )
                    nc.tensor.matmul(
                        l0ps[32:33, :], mke[:, kc : kc + 1], wt[:],
                        start=(kc == 0), stop=(kc == 1),
                    )
                nc.scalar.activation(
                    em3b[32:33, sl], l0ps[32:33, :], AF.Copy, scale=DELTA
                )

            # S0 = log(sum(exp(L0)))  (L0 tiny => no max subtraction needed)
            etrash = embp.tile([33, N], BF16)
            acc33 = cst.tile([33, 1], F32)
            nc.scalar.activation(
                etrash[32:33, :], em3b[32:33, :], AF.Exp,
                scale=1.0 / DELTA, accum_out=acc33[32:33, :],
            )
            s0t = cst.tile([33, 1], F32)
            nc.scalar.activation(s0t[32:33, :], acc33[32:33, :], AF.Ln)

            # masks -> m_sb [128, NT] fp32
            pm8 = sm.tile([128, NT], U8)
            mm8 = sm.tile([128, NT], U8)
            nc.sync.dma_start(pm8[:], pm.rearrange("(a b) -> b a", b=128))
            nc.sync.dma_start(mm8[:], mm.rearrange("(a b) -> b a", b=128))
            pmf = sm.tile([128, NT], F32)
            mmf = sm.tile([128, NT], F32)
            nc.vector.tensor_copy(pmf[:], pm8[:])
            nc.vector.tensor_copy(mmf[:], mm8[:])
            m_sb = cst.tile([128, NT], F32)
            nc.vector.tensor_tensor(m_sb[:], pmf[:], mmf[:], op=OP.mult)

            numacc = cst.tile([128, NT], F32)

            # ---------------- per token-tile main loop ----------------
            for i in range(NT):
                x_t = xsp.tile([128, D], F32)
                nc.sync.dma_start(x_t[:], xs[i * 128 : (i + 1) * 128, :])

                ssum = sm.tile([128, 1], F32)
                nc.vector.tensor_reduce(ssum[:], x_t[:], axis=AX, op=OP.add)
                mu = sm.tile([128, 1], F32)
                nc.vector.tensor_scalar(mu[:], ssum[:], 1.0 / D, None, op0=OP.mult)
                xc = wk.tile([128, D], F32)
                nc.vector.tensor_scalar(xc[:], x_t[:], mu[:], None, op0=OP.subtract)

                sqt = wk.tile([128, D], F32)
                ssq = sm.tile([128, 1], F32)
                nc.scalar.activation(sqt[:], xc[:], AF.Square, accum_out=ssq[:])
                # rstd = exp(-0.5*ln(var+eps)) — keeps ACT inside the
                # natural_log_exp table set (Sqrt would thrash table loads)
                lnv = sm.tile([128, 1], F32)
                nc.scalar.activation(
                    lnv[:], ssq[:], AF.Ln, scale=1.0 / D, bias=epsb[:]
                )
                rstd = sm.tile([128, 1], F32)
                nc.scalar.activation(rstd[:], lnv[:], AF.Exp, scale=-0.5)
                z = wk.tile([128, D], F32)
                nc.vector.tensor_scalar(z[:], xc[:], rstd[:], None, op0=OP.mult)

                # zT (D on partitions) via PE transpose
                zt = wk.tile([128, 2, 128], F32)
                for kc in range(2):
                    tp = psm.tile([128, 128], F32, tag="misc")
                    nc.tensor.transpose(
                        tp[:], z[:, kc * 128 : (kc + 1) * 128], ident[:]
                    )
                    nc.vector.tensor_copy(zt[:, kc, :], tp[:])

                # projT [E, 128] + bias b0t; build bf16 lhsT33
                ppj = psm.tile([E, 128], F32, tag="misc")
                for kc in range(2):
                    nc.tensor.matmul(
                        ppj[:], pp[:, kc, :], zt[:, kc, :],
                        start=(kc == 0), stop=(kc == 1),
                    )
                lhs = wk.tile([33, 128], BF16)
                nc.vector.memset(lhs[0:32, :], -0.5)
                nc.vector.tensor_scalar(
                    lhs[0:E, :], ppj[:], b0t[:], None, op0=OP.add
                )
                nc.vector.memset(lhs[32:33, :], 1.0)

                maxs_c = sm.tile([128, NBLK], F32)
                vsum_c = sm.tile([128, NBLK], F32)

                # phase A: score matmuls, DVE max-scan straight from psum
                for g in range(NBLK):
                    pa = psc.tile([128, BLK], F32, tag="blk")
                    for h in range(BLK // 512):
                        sl = slice(g * BLK + h * 512, g * BLK + (h + 1) * 512)
                        nc.tensor.matmul(
                            pa[:, h * 512 : (h + 1) * 512],
                            lhs[0:32, :], em3b[0:32, sl],
                            start=True, stop=True,
                        )
                    nc.vector.tensor_reduce(
                        maxs_c[:, g : g + 1], pa[:], axis=AX, op=OP.max
                    )
                maxs = sm.tile([128, 1], F32)
                nc.vector.tensor_reduce(maxs[:], maxs_c[:], axis=AX, op=OP.max)
                nbm = sm.tile([128, 1], F32)
                nc.vector.tensor_scalar(
                    nbm[:], maxs[:], -BETA, None, op0=OP.mult
                )

                # phase B: v matmuls; ACT does exp(beta*(v-maxs)) + sum-accum
                # straight from psum. ln(sum) ~= beta*delta*L0[argmax score].
                for g in range(NBLK):
                    pb = psc.tile([128, BLK], F32, tag="blk")
                    for h in range(BLK // 512):
                        sl = slice(g * BLK + h * 512, g * BLK + (h + 1) * 512)
                        nc.tensor.matmul(
                            pb[:, h * 512 : (h + 1) * 512],
                            lhs[0:33, :], em3b[0:33, sl],
                            start=True, stop=True,
                        )
                    etr = vsbp.tile([128, BLK], BF16)
                    nc.scalar.activation(
                        etr[:], pb[:], AF.Exp, scale=BETA, bias=nbm[:],
                        accum_out=vsum_c[:, g : g + 1],
                    )

                vsum = sm.tile([128, 1], F32)
                nc.vector.tensor_reduce(vsum[:], vsum_c[:], axis=AX, op=OP.add)
                dl0 = sm.tile([128, 1], F32)
                nc.scalar.activation(dl0[:], vsum[:], AF.Ln)
                nc.vector.tensor_tensor(
                    numacc[:, i : i + 1], dl0[:], m_sb[:, i : i + 1], op=OP.mult
                )

            # ---------------- finalize ----------------
            pair = cst.tile([128, 2], F32)
            nc.vector.tensor_reduce(pair[:, 0:1], numacc[:], axis=AX, op=OP.add)
            nc.vector.tensor_reduce(pair[:, 1:2], m_sb[:], axis=AX, op=OP.add)
            pps = psm.tile([2, 1], F32, tag="misc")
            nc.tensor.matmul(pps[:], pair[:], ones128[:], start=True, stop=True)
            pout = cst.tile([2, 1], F32)
            nc.vector.tensor_scalar(pout[:], pps[:], scale2[:], None, op0=OP.mult)
            nc.sync.dma_start(out[0:2, :], pout[:])
            nc.sync.dma_start(out[2:3, :], s0t[32:33, :])

    nc.finalize()
    return nc


def _prep_in_maps(xs, pad_mask, masked_masks, ln_gamma, ln_beta, projection,
                  embeddings, top_n_out, mask_emb):
    xsf = np.ascontiguousarray(np.asarray(xs, np.float32).reshape(B * T, D))
    pmf = np.asarray(pad_mask).reshape(-1).astype(bool)
    mmf = np.asarray(masked_masks).reshape(-1).astype(bool)

    shared = {
        "gamma": np.ascontiguousarray(np.asarray(ln_gamma, np.float32)),
        "beta": np.ascontiguousarray(np.asarray(ln_beta, np.float32)),
        "projw": np.ascontiguousarray(np.asarray(projection, np.float32)),
        "emb": np.ascontiguousarray(np.asarray(embeddings, np.float32)[0]),
        "wmat": np.ascontiguousarray(np.asarray(top_n_out, np.float32)[0]),
        "maske": np.ascontiguousarray(np.asarray(mask_emb, np.float32)),
    }

    # only tokens with pad & masked contribute; gather and spread across cores
    sel = np.nonzero(pmf & mmf)[0]
    assert len(sel) <= NCORES * TOK, (
        f"masked token count {len(sel)} exceeds static capacity {NCORES * TOK}"
    )
    chunks = np.array_split(sel, NCORES)
    in_maps = []
    for c in range(NCORES):
        idx = chunks[c]
        n = len(idx)
        xs_c = np.zeros((TOK, D), np.float32)
        pm_c = np.zeros((TOK,), np.uint8)
        mm_c = np.zeros((TOK,), np.uint8)
        if n:
            xs_c[:n] = xsf[idx]
            pm_c[:n] = pmf[idx]
            mm_c[:n] = mmf[idx]
        in_maps.append({"xs": xs_c, "pm": pm_c, "mm": mm_c, **shared})
    return in_maps


def kernel(**inputs) -> np.ndarray:
    if "nc" not in _CACHE:
        _CACHE["nc"] = _build_bass()
    nc = _CACHE["nc"]
    in_maps = _prep_in_maps(**inputs)
    res = bass_utils.run_bass_kernel_spmd(nc, in_maps, core_ids=list(range(NCORES)))
    num = 0.0
    cnt = 0.0
    s0 = None
    for r in res.results:
        o = r["out"].reshape(3)
        num += float(o[0])
        cnt += float(o[1])
        s0 = float(o[2])
    loss = np.float32(s0 - num / cnt)
    return np.asarray(loss, np.float32)



# revision 13
# speedup vs baseline: 1.4083x; 1.4083x over previous
"""BestRQ loss kernel for 8 Trainium2 NeuronCores (v2, single-pass blocks).

Math (exact reformulations of the reference):
  - loss = S0 - (sum_t m_t * L0[target_t]) / sum(m), with
    L0 = mask_emb @ W (shared logits row at every masked token) and
    S0 = logsumexp(L0).  Only masked tokens contribute.
  - target_t = argmax_n score_tn, score_tn = proj_t . emb_n - 0.5|emb_n|^2.
  - L0[target_t] extracted without indices: per 1024-code block g,
        bmax_g = max_n score                       (DVE reduce from PSUM)
        psum  += delta*L0                          (K=1 accum matmul, row17)
        vsum_g = sum_n exp(beta*(v - bmax_g))      (ACT exp + accum)
    then vtot = sum_g vsum_g * exp(beta*(bmax_g - gmax)) ~= exp(beta*delta*
    L0[argmax]), dl0 = ln(vtot) = beta*delta*L0[target].
  - 4096 masked tokens -> 4 tiles x 128 per core; the <=128 leftovers are
    replicated on every core as a "tail" tile where each core scores only its
    own 1/8 of the codebook (per-core block-rotated codebook; argmax is
    column-order invariant) and the host combines the per-core partials.
  - W is shipped fp8e4 and streamed once to build delta*L0 (16 matmuls into
    col-group-packed PSUM rows, M=32 replication so the row escapes PSUM in
    one full-width ACT copy); S0 comes from that row reshaped to [128,64].
"""

import math

import numpy as np

try:
    import concourse.bass as bass  # noqa: F401
except ImportError:  # pragma: no cover
    import sys

    sys.path.insert(0, "/opt/trn_rl_repo")
    import concourse.bass as bass  # noqa: F401

import concourse.mybir as mybir
from concourse import bacc, bass_utils, masks
from concourse.tile import TileContext

F32 = mybir.dt.float32
BF16 = mybir.dt.bfloat16
FP8 = mybir.dt.float8e4

B, T, D, E, N = 16, 512, 256, 16, 8192
NCORES = 8
EPS = 1e-5
DELTA = 1e-2
BETA = 2000.0
NBLK = 8          # 1024-code blocks
BLK = N // NBLK

_CACHE = {}


def _build_bass(NT, use_tail):
    nc = bacc.Bacc(
        "TRN2", target_bir_lowering=False, debug=False, num_devices=NCORES
    )
    NLN = NT + (1 if use_tail else 0)
    xsm = nc.dram_tensor("xsm", [128, NT, D], F32, kind="ExternalInput")
    xst = nc.dram_tensor("xst", [128, D], F32, kind="ExternalInput")
    mmain = nc.dram_tensor("mmain", [128, NT], F32, kind="ExternalInput")
    embb = nc.dram_tensor("embb", [E, N], BF16, kind="ExternalInput")
    qrow = nc.dram_tensor("qrow", [1, N], BF16, kind="ExternalInput")
    ppb = nc.dram_tensor("ppb", [128, 2, E], BF16, kind="ExternalInput")
    b0t = nc.dram_tensor("b0t", [E, 1], F32, kind="ExternalInput")
    mk2 = nc.dram_tensor("mk2", [128, 2, 32], FP8, kind="ExternalInput")
    wb = nc.dram_tensor("wb", [NBLK, 128, 2, 1024], FP8, kind="ExternalInput")
    out = nc.dram_tensor("out", [258, 1], F32, kind="ExternalOutput")

    AX = mybir.AxisListType.X
    OP = mybir.AluOpType
    AF = mybir.ActivationFunctionType

    with TileContext(nc) as tc:
        with (
            tc.tile_pool(name="cst", bufs=1) as cst,
            tc.tile_pool(name="xsp", bufs=1) as xsp,
            tc.tile_pool(name="wp", bufs=3) as wp,
            tc.tile_pool(name="lnp", bufs=2) as lnp,
            tc.tile_pool(name="lhp", bufs=NLN) as lhp,
            tc.tile_pool(name="smp", bufs=2) as smp,
            tc.tile_pool(name="ps", bufs=3, space="PSUM") as ps,
            tc.tile_pool(name="psm", bufs=1, space="PSUM") as psm,
        ):
            # ---------------- constants / big DMAs ----------------
            em18 = cst.tile([17, N], BF16)
            row17 = cst.tile([1, N], BF16)
            nc.sync.dma_start(em18[0:16, :], embb[:, :])
            nc.sync.dma_start(em18[16:17, :], qrow[:, :])

            mrow = cst.tile([128, NT], F32)
            nc.sync.dma_start(mrow[:], mmain[:, :])
            xall = xsp.tile([128, NT, D], F32)
            nc.sync.dma_start(xall[:], xsm[:, :, :])
            xtail = xsp.tile([128, D], F32)
            if use_tail:
                nc.sync.dma_start(xtail[:], xst[:, :])

            pp = cst.tile([128, 2, E], BF16)
            nc.sync.dma_start(pp[:], ppb[:, :, :])
            b0 = cst.tile([E, 1], F32)
            nc.sync.dma_start(b0[:], b0t[:, :])
            mk = cst.tile([128, 2, 32], FP8)
            nc.sync.dma_start(mk[:], mk2[:, :, :])

            ident = cst.tile([128, 128], BF16)
            masks.make_identity(nc, ident[:])

            ones1 = cst.tile([1, 128], BF16)
            nc.vector.memset(ones1[:], 1.0)
            ones128 = cst.tile([128, 1], F32)
            nc.vector.memset(ones128[:], 1.0)

            epsb = cst.tile([128, 1], F32)
            nc.vector.memset(epsb[:], EPS)

            etr = cst.tile([128, BLK], BF16)       # exp trash output
            dl0rep = cst.tile([128, 1024], BF16)   # delta*L0, 32x-replicated
            s0t = cst.tile([128, 64], BF16)
            s0acc = cst.tile([128, 1], F32)
            numacc = cst.tile([128, NT], F32)

            # ---------------- building blocks ----------------
            def ln_tile(i):
                """LayerNorm -> projected lhs [17,128] for tile i."""
                x_t = xall[:, i, :] if i < NT else xtail[:]
                st6 = lnp.tile([128, 6], F32, tag="st6")
                nc.vector.bn_stats(st6[:], x_t)
                mv = lnp.tile([128, 2], F32, tag="mv")
                nc.vector.bn_aggr(mv[:], st6[:])
                lnv = lnp.tile([128, 1], F32, tag="lnv")
                nc.scalar.activation(lnv[:], mv[:, 1:2], AF.Ln, bias=epsb[:])
                rstd = lnp.tile([128, 1], F32, tag="rstd")
                nc.scalar.activation(rstd[:], lnv[:], AF.Exp, scale=-0.5)
                z = lnp.tile([128, D], BF16, tag="z")
                nc.gpsimd.tensor_scalar(
                    z[:], x_t, mv[:, 0:1], rstd[:],
                    op0=OP.subtract, op1=OP.mult,
                )
                mtz = psm.tile([128, 1024], F32, tag="misc")
                ztp = mtz[:].bitcast(BF16)[:, 0:256]
                for kc in range(2):
                    nc.tensor.transpose(
                        ztp[:, kc * 128:(kc + 1) * 128],
                        z[:, kc * 128:(kc + 1) * 128], ident[:],
                    )
                zt = lnp.tile([128, 2, 128], BF16, tag="ztsb")
                nc.scalar.activation(zt[:, 0, :], ztp[:, 0:128], AF.Copy)
                nc.scalar.activation(zt[:, 1, :], ztp[:, 128:256], AF.Copy)
                mtp = psm.tile([128, 1024], F32, tag="misc")
                ppj = mtp[0:16, 0:128]
                for kc in range(2):
                    nc.tensor.matmul(
                        ppj, pp[:, kc, :], zt[:, kc, :],
                        start=(kc == 0), stop=(kc == 1),
                    )
                lhs = lhp.tile([17, 128], BF16, tag="lhs")
                nc.vector.memset(lhs[:], 1.0)
                nc.vector.tensor_scalar(
                    lhs[0:16, :], ppj, b0[:], None, op0=OP.add
                )
                return lhs

            def psl_batch(b):
                """fp8 W chunks 4b..4b+3 -> delta*L0 row cols [4096b:...]."""
                psl = psm.tile([128, 1024], F32, tag="misc", name="psl")[:]
                for cg in range(4):
                    g = 4 * b + cg
                    wt = wp.tile([128, 2, 1024], FP8, tag="wt")
                    nc.sync.dma_start(wt[:], wb[g, :, :, :])
                    for h in range(2):
                        hs = slice(h * 512, (h + 1) * 512)
                        for dc in range(2):
                            nc.tensor.matmul(
                                psl[32 * cg:32 * cg + 32, hs],
                                mk[:, dc, :], wt[:, dc, hs],
                                start=(dc == 0), stop=(dc == 1),
                                tile_position=(0, 32 * cg),
                            )
                nc.scalar.activation(dl0rep[:, :], psl, AF.Copy, scale=DELTA)
                src = dl0rep[:].rearrange("(c s) j -> c s j", s=32)[:, 0:1, :]
                nc.sync.dma_start(row17[0:1, b * 4096:(b + 1) * 4096], src)

            def do_block(lhs, g, bmax, nbb, vsum):
                pt = ps.tile([128, BLK], F32, tag="pair")
                for h in range(2):
                    hs = slice(g * BLK + h * 512, g * BLK + (h + 1) * 512)
                    nc.tensor.matmul(
                        pt[:, h * 512:(h + 1) * 512], lhs[0:17, :],
                        em18[0:17, hs], start=True, stop=True,
                    )
                nc.vector.tensor_reduce(
                    bmax[:, g:g + 1], pt[:], axis=AX, op=OP.max
                )
                nc.gpsimd.tensor_scalar(
                    nbb[:, g:g + 1], bmax[:, g:g + 1], -BETA, None,
                    op0=OP.mult,
                )
                for h in range(2):
                    hs = slice(g * BLK + h * 512, g * BLK + (h + 1) * 512)
                    nc.tensor.matmul(
                        pt[:, h * 512:(h + 1) * 512], ones1[:],
                        row17[0:1, hs], start=False, stop=True,
                        skip_group_check=True,
                    )
                nc.scalar.activation(
                    etr[:], pt[:], AF.Exp, scale=BETA,
                    bias=nbb[:, g:g + 1], accum_out=vsum[:, g:g + 1],
                )

            def combine_tile(i, bmax, vsum):
                gmax = smp.tile([128, 1], F32, tag="gmax")
                nc.vector.tensor_reduce(gmax[:], bmax[:], axis=AX, op=OP.max)
                ngm = smp.tile([128, 1], F32, tag="ngm")
                nc.gpsimd.tensor_scalar(
                    ngm[:], gmax[:], -BETA, None, op0=OP.mult
                )
                wg = smp.tile([128, NBLK], F32, tag="wg")
                nc.scalar.activation(
                    wg[:], bmax[:], AF.Exp, scale=BETA, bias=ngm[:]
                )
                vd = smp.tile([128, NBLK], F32, tag="vd")
                nc.gpsimd.tensor_tensor(vd[:], vsum[:], wg[:], op=OP.mult)
                vtot = smp.tile([128, 1], F32, tag="vtot")
                nc.vector.tensor_reduce(vtot[:], vd[:], axis=AX, op=OP.add)
                dl0 = smp.tile([128, 1], F32, tag="dl0")
                nc.scalar.activation(dl0[:], vtot[:], AF.Ln)
                nc.gpsimd.tensor_tensor(
                    numacc[:, i:i + 1], dl0[:], mrow[:, i:i + 1], op=OP.mult
                )

            def tile_state():
                bmax = smp.tile([128, NBLK], F32, tag="bmax")
                nbb = smp.tile([128, NBLK], F32, tag="nbb")
                vsum = smp.tile([128, NBLK], F32, tag="vsum")
                return bmax, nbb, vsum

            # ---------------- emission schedule ----------------
            # LN(0), LN(1) first so tile0 can start while W streams in.
            lhs0 = ln_tile(0)
            lhs1 = ln_tile(1) if NT > 1 else None

            psl_batch(0)                       # row17 cols 0:4096
            st0 = tile_state()
            for g in range(4):
                do_block(lhs0, g, *[st0[k] for k in (0, 1, 2)])
            psl_batch(1)                       # row17 cols 4096:8192
            lhs_rest = [ln_tile(i) for i in range(2, NLN)]
            for g in range(4, NBLK):
                do_block(lhs0, g, *[st0[k] for k in (0, 1, 2)])
            combine_tile(0, st0[0], st0[2])

            # S0 = ln sum exp(L0): row17 -> [128,64] -> exp-accum
            nc.sync.dma_start(s0t[:], row17[0:1, :])
            nc.scalar.activation(
                etr[:, 0:64], s0t[:], AF.Exp, scale=1.0 / DELTA,
                accum_out=s0acc[:],
            )

            all_lhs = [lhs0] + ([lhs1] if lhs1 is not None else []) + lhs_rest
            for i in range(1, NT):
                sti = tile_state()
                for g in range(NBLK):
                    do_block(all_lhs[i], g, *[sti[k] for k in (0, 1, 2)])
                combine_tile(i, sti[0], sti[2])

            if use_tail:
                stt = tile_state()
                do_block(all_lhs[NT], 0, *[stt[k] for k in (0, 1, 2)])
                nc.sync.dma_start(out[2:130, :], stt[0][:, 0:1])
                nc.sync.dma_start(out[130:258, :], stt[2][:, 0:1])

            # ---------------- finale ----------------
            pair2 = cst.tile([128, 2], F32)
            nc.vector.tensor_reduce(
                pair2[:, 0:1], numacc[:], axis=AX, op=OP.add
            )
            nc.vector.tensor_copy(pair2[:, 1:2], s0acc[:])
            finps = psm.tile([128, 1024], F32, tag="misc", name="finb")[0:2, 0:1]
            nc.tensor.matmul(
                finps, pair2[:], ones128[:], start=True, stop=True
            )
            outsb = cst.tile([2, 1], F32)
            nc.scalar.activation(outsb[:], finps, AF.Copy)
            nc.sync.dma_start(out[0:2, :], outsb[:])

    nc.finalize()
    return nc


def _prep_in_maps(xs, pad_mask, masked_masks, ln_gamma, ln_beta, projection,
                  embeddings, top_n_out, mask_emb):
    import ml_dtypes

    xsf = np.ascontiguousarray(np.asarray(xs, np.float32).reshape(B * T, D))
    pmf = np.asarray(pad_mask).reshape(-1).astype(bool)
    mmf = np.asarray(masked_masks).reshape(-1).astype(bool)
    gamma = np.asarray(ln_gamma, np.float32)
    beta = np.asarray(ln_beta, np.float32)
    proj = np.asarray(projection, np.float32)
    emb = np.asarray(embeddings, np.float32)[0]          # [E, N]
    wmat = np.asarray(top_n_out, np.float32)[0]          # [D, N]
    maske = np.asarray(mask_emb, np.float32)

    sel = np.nonzero(pmf & mmf)[0]
    n = len(sel)
    NT = max(1, -(-max(n - 128, 1) // (NCORES * 128)))
    nmain = min(n, NCORES * 128 * NT)
    L = n - nmain
    assert L <= 128, f"tail overflow: {L}"
    use_tail = L > 0

    main_idx = sel[:nmain]
    xs_cores, m_cores = [], []
    for c in range(NCORES):
        idx = main_idx[c * 128 * NT:(c + 1) * 128 * NT]
        k = len(idx)
        xc = np.zeros((NT * 128, D), np.float32)
        mc = np.zeros((NT * 128,), np.float32)
        if k:
            xc[:k] = xsf[idx]
            mc[:k] = 1.0
        xs_cores.append(
            np.ascontiguousarray(xc.reshape(NT, 128, D).transpose(1, 0, 2))
        )
        m_cores.append(
            np.ascontiguousarray(mc.reshape(NT, 128).transpose(1, 0))
        )

    xt = np.zeros((128, D), np.float32)
    if use_tail:
        xt[:L] = xsf[sel[nmain:]]

    # gamma folded into projection, beta into the projected bias (host fold)
    ppf = (gamma[:, None] * proj).astype(np.float32)     # [D, E]
    b0 = (beta @ proj).astype(np.float32).reshape(E, 1)  # [E, 1]
    ppb = np.ascontiguousarray(
        ppf.reshape(2, 128, E).transpose(1, 0, 2)
    ).astype(ml_dtypes.bfloat16)

    # mk2[p, dc, j] = maske[dc*128 + p], replicated over 32 cols (M=32)
    mk2 = np.ascontiguousarray(
        np.repeat(maske.reshape(2, 128).T[:, :, None], 32, axis=2)
    ).astype(ml_dtypes.float8_e4m3fn)

    in_maps = []
    for c in range(NCORES):
        perm = np.roll(np.arange(N), -c * 1024)
        embP = emb[:, perm]
        wP = wmat[:, perm]
        qrowP = (-0.5 * np.sum(embP.astype(np.float64) ** 2, axis=0)).astype(
            np.float32
        ).reshape(1, N)
        wbP = np.ascontiguousarray(
            wP.reshape(2, 128, NBLK, 1024).transpose(2, 1, 0, 3)
        )
        in_maps.append({
            "xsm": xs_cores[c],
            "xst": xt,
            "mmain": m_cores[c],
            "embb": np.ascontiguousarray(embP).astype(ml_dtypes.bfloat16),
            "qrow": qrowP.astype(ml_dtypes.bfloat16),
            "ppb": ppb,
            "b0t": b0,
            "mk2": mk2,
            "wb": wbP.astype(ml_dtypes.float8_e4m3fn),
        })
    return in_maps, NT, use_tail, n, L


def kernel(**inputs) -> np.ndarray:
    in_maps, NT, use_tail, n, L = _prep_in_maps(**inputs)
    key = (NT, use_tail)
    if key not in _CACHE:
        _CACHE[key] = _build_bass(NT, use_tail)
        _CACHE["nc"] = _CACHE[key]
    nc = _CACHE[key]
    res = bass_utils.run_bass_kernel_spmd(
        nc, in_maps, core_ids=list(range(NCORES))
    )
    num = 0.0
    s0sum = None
    bmt = np.zeros((NCORES, 128), np.float64)
    vst = np.zeros((NCORES, 128), np.float64)
    for c, r in enumerate(res.results):
        o = np.asarray(r["out"], np.float64).reshape(258)
        num += float(o[0])
        s0sum = float(o[1])
        bmt[c] = o[2:130]
        vst[c] = o[130:258]
    if L > 0:
        gm = bmt.max(axis=0)
        w = np.exp(BETA * (bmt - gm[None, :]))
        vtot = (vst * w).sum(axis=0)
        num += float(np.log(vtot[:L]).sum())
    s0 = math.log(s0sum)
    loss = np.float32(s0 - num / (BETA * DELTA) / n)
    return np.asarray(loss, np.float32)
